# revision 2
# baseline (speedup 1.0000x reference)
"""Trainium2 Bass kernel for nn_DctAtt (B=32, D=1024, N=4096, K=5).

The reference computes, per (b, d) row of x:
    coeffs = x[b,d,:] @ C          (C: [N, K] DCT-II ortho, first K rows)
    att    = coeffs @ dw_w + dw_b
Both steps are linear in x, so they collapse into a single dot product
with the precomputed vector w = C @ dw_w:
    att[b,d] = x[b,d,:] . w + dw_b
The device kernel streams x (512 MiB total) through that dot product --
this is the memory-bound part. The remaining work (BatchNorm over all
B*D values, GELU, scalar affine, softmax over D) touches only a
[32, 1024] array and runs on the host, using the exact global batch
statistics (no per-device approximation).

Sharding: data-parallel over batch B across the 8 NeuronCores
(4 batches = 4096 rows of 4096 floats = 64 MiB per core).

Per-core kernel: 32 tiles of [128 rows, 4096]. Each tile is one 2 MiB
contiguous HWDGE DMA plus one fused DVE tensor_tensor_reduce
(out = x*w, accum = row-sum) producing a [128, 1] column of partial
dot products. DMA is the bottleneck (~5.8us/tile at ~358 GB/s HBM
vs ~4.4us/tile on DVE), so the kernel runs at the memory roofline.
"""

import math

import numpy as np

import concourse.bacc as bacc
import concourse.mybir as mybir
import concourse.tile as tile
from concourse import bass_utils

# Problem constants (hardcoded: the grading harness ships only this file).
B, D, N = 32, 1024, 4096
K = 5
BN_EPS = 1e-5
N_CORES = 8
P = 128
ROWS_PER_CORE = (B // N_CORES) * D  # 4096
N_TILES = ROWS_PER_CORE // P  # 32
XP_BUFS = 6  # in-flight x tiles (2 MiB each)

_compiled_nc = None


def _build():
    """Build + compile the per-core Bass program (cached per process)."""
    global _compiled_nc
    if _compiled_nc is not None:
        return _compiled_nc

    nc = bacc.Bacc(
        "TRN2",
        target_bir_lowering=False,
        debug=False,
        enable_asserts=False,
        num_devices=N_CORES,
    )
    f32 = mybir.dt.float32
    x_sh = nc.dram_tensor("x_sh", [ROWS_PER_CORE, N], f32, kind="ExternalInput").ap()
    w_rep = nc.dram_tensor("w_rep", [P, N], f32, kind="ExternalInput").ap()
    y_out = nc.dram_tensor("y_out", [P, N_TILES], f32, kind="ExternalOutput").ap()

    with tile.TileContext(nc) as tc:
        with (
            tc.tile_pool(name="wp", bufs=1) as wp,
            tc.tile_pool(name="xp", bufs=XP_BUFS) as xp,
            tc.tile_pool(name="sp", bufs=1) as sp,
            tc.tile_pool(name="yp", bufs=1) as yp,
        ):
            w_sb = wp.tile([P, N], f32)
            nc.sync.dma_start(out=w_sb, in_=w_rep)
            y_sb = yp.tile([P, N_TILES], f32)
            # Stride-0 free dim: the fused op's elementwise product is not
            # materialised (every element lands on the same column).
            dummy = sp.tile([P, 1], f32)
            xv = x_sh.rearrange("(t p) n -> t p n", p=P)
            for t in range(N_TILES):
                xt = xp.tile([P, N], f32)
                nc.sync.dma_start(out=xt, in_=xv[t])
                # accum = sum((x*1 + 0) * w) per partition = row dot product.
                nc.vector.affine_mul_reduce(
                    out=dummy.broadcast_to(xt.shape),
                    accum_out=y_sb[:, t : t + 1],
                    in0=xt,
                    in1=w_sb,
                    scale=1.0,
                    bias=0.0,
                )
            nc.sync.dma_start(out=y_out, in_=y_sb)

    nc.compile()
    _compiled_nc = nc
    return nc


def _dct_weight(dw_w):
    """w = C @ dw_w in float64, where C is the [N, K] ortho DCT-II basis."""
    n = np.arange(N, dtype=np.float64)
    k = np.arange(K, dtype=np.float64)
    C = np.cos(np.pi * (2.0 * n[:, None] + 1.0) * k[None, :] / (2.0 * N))
    C *= math.sqrt(2.0 / N)
    C[:, 0] *= 1.0 / math.sqrt(2.0)
    return (C @ np.asarray(dw_w, dtype=np.float64)).astype(np.float32)


def _erf(x):
    try:
        from scipy.special import erf

        return erf(x)
    except Exception:
        return np.vectorize(math.erf)(x).astype(x.dtype)


def _run_device(inputs, trace=False, **spmd_kwargs):
    """Run the dot-product phase on the 8 cores; return att [B, D] (pre-BN)
    and the BassKernelResults (for profiling from test harnesses)."""
    x = np.ascontiguousarray(np.asarray(inputs["x"], dtype=np.float32))
    w = _dct_weight(inputs["dw_w"])
    w_rep = np.ascontiguousarray(np.broadcast_to(w[None, :], (P, N)))

    nc = _build()
    b_per_core = B // N_CORES
    in_maps = []
    for c in range(N_CORES):
        xs = np.ascontiguousarray(
            x[c * b_per_core : (c + 1) * b_per_core].reshape(ROWS_PER_CORE, N)
        )
        in_maps.append({"x_sh": xs, "w_rep": w_rep})

    res = bass_utils.run_bass_kernel_spmd(
        nc, in_maps, core_ids=list(range(N_CORES)), trace=trace, **spmd_kwargs
    )
    # y_out[p, t] = shard row t*128 + p  ->  transpose+ravel restores row order.
    att = np.concatenate(
        [res.results[c]["y_out"].T.reshape(-1) for c in range(N_CORES)]
    ).reshape(B, D)
    return att, res


def _postprocess(att, inputs):
    """Host tail on the tiny [B, D] array: +dw_b, BatchNorm (global batch
    stats, training mode), exact GELU, 1x1 conv affine, softmax over D."""
    dw_b = np.float32(np.asarray(inputs["dw_b"]).reshape(-1)[0])
    gamma = np.float32(np.asarray(inputs["gamma"]).reshape(-1)[0])
    beta = np.float32(np.asarray(inputs["beta"]).reshape(-1)[0])
    conv_w = np.float32(np.asarray(inputs["conv_w"]).reshape(-1)[0])
    conv_b = np.float32(np.asarray(inputs["conv_b"]).reshape(-1)[0])

    att = att.astype(np.float32) + dw_b
    mean = att.mean(dtype=np.float64)
    var = np.mean((att.astype(np.float64) - mean) ** 2)
    inv_std = np.float32(1.0 / math.sqrt(var + BN_EPS))
    att = (att - np.float32(mean)) * inv_std * gamma + beta
    # Exact GELU: x * 0.5 * (1 + erf(x / sqrt(2)))
    att = (att * 0.5 * (1.0 + _erf(att / np.float32(math.sqrt(2.0))))).astype(
        np.float32
    )
    att1 = att * conv_w + conv_b
    att1 = att1 - att1.max(axis=-1, keepdims=True)
    e = np.exp(att1.astype(np.float32))
    att1 = (e / e.sum(axis=-1, keepdims=True)).astype(np.float32)
    att1 = att1[:, :, None]
    return att1, (np.float32(1.0) - att1).astype(np.float32)


def kernel(**inputs):
    att, _ = _run_device(inputs)
    return _postprocess(att, inputs)
